# revision 60
# baseline (speedup 1.0000x reference)
"""MinGRU layer kernel for 8 Trainium2 NeuronCores.

Problem: x (4, 8192, 1024) f32; Wz, Wh (1024, 1024); bz, bh (1024,)
    z = sigmoid(x @ Wz + bz); h_tilde = x @ Wh + bh
    h_t = (1 - z_t) * h_{t-1} + z_t * h_tilde_t   (scan over seq, h_{-1} = 0)

Sharding: 8 cores = 4 batches x 2 output-dim halves. The scan is
independent per (batch, dim), so each core owns a full-sequence scan for
one batch and 512 of the 1024 output dims -- no cross-core traffic.

Layout: host pre-transposes x to (d_in, seq) fp16 per batch. On device the
matmul keeps W stationary (lhsT = W tile, natural layout) and streams x^T,
producing (d_out, seq) tiles in PSUM -- exactly the layout
tensor_tensor_scan needs (scan runs along the free/seq axis, one recurrence
per partition/dim). ScalarE computes a = sigmoid(-(z_pre)) and
z = sigmoid(z_pre) straight out of PSUM; VectorE fuses b = (h_pre + bh) * z
and then runs the scan. Output h^T (512, 8192) f32 is written contiguously;
the host transposes back during the gather.
"""

import sys

if "/opt/trn_rl_repo" not in sys.path:
    sys.path.insert(0, "/opt/trn_rl_repo")

import numpy as np

from concourse import bass, mybir
from concourse.tile import TileContext
from concourse.bass_utils import run_bass_kernel_spmd

BATCH, SEQ, D = 4, 8192, 1024
DH = 512            # output dims per core
N_CORES = 8
# Seq chunk schedule: small chunks first so the PE starts on real work
# early (warms the HAM clock gate) and the consumer engines ramp before
# the PE hits full streaming rate.
CHUNKS = [256, 256, 512] + [1024] * 6 + [512, 256, 128, 128]
assert sum(CHUNKS) == SEQ
NCHUNK = len(CHUNKS)
CHUNK_MAX = max(CHUNKS)
NM = DH // 128      # output-dim tiles per core
NK = D // 128       # contraction tiles

F16 = mybir.dt.float16
F32 = mybir.dt.float32
AF = mybir.ActivationFunctionType
OP = mybir.AluOpType


_WAIT_LIMIT = 1  # this walrus build rejects multiple sem waits per instruction


def _split_sync_waits(nc):
    """Move excess semaphore waits (beyond _WAIT_LIMIT) off each instruction
    onto same-engine nops inserted immediately before it. Waits only gate
    execution, so hoisting some onto a preceding nop in the same engine
    stream is semantics-preserving."""
    import bass_rust

    n_extra = 0
    for fn in nc.m.functions:
        for blk in fn.blocks:
            insts = blk.instructions
            out = []
            for inst in insts:
                si = inst.sync_info
                if si is not None and si.on_wait and len(si.on_wait) > _WAIT_LIMIT:
                    waits = list(si.on_wait)
                    head, tail = waits[:-_WAIT_LIMIT], waits[-_WAIT_LIMIT:]
                    for j in range(0, len(head), _WAIT_LIMIT):
                        n_extra += 1
                        nop = bass_rust.InstNoOp(
                            name=f"{inst.name}-waitsplit{j}",
                            engine=inst.engine,
                            sync_info=type(si)(
                                on_wait=head[j:j + _WAIT_LIMIT], on_update=[]
                            ),
                            bass_nofuse=True,
                        )
                        nc.register_instruction(nop, overwrite=True)
                        out.append(nop)
                    si.on_wait = tail
                out.append(inst)
            if n_extra:
                blk.instructions = out
    return n_extra


def _build_program():
    nc = bass.Bass("TRN2", target_bir_lowering=False, debug=False)

    xT = nc.dram_tensor("xT", [D, SEQ], F16, kind="ExternalInput").ap()
    wz = nc.dram_tensor("wz", [D, DH], F16, kind="ExternalInput").ap()
    wh = nc.dram_tensor("wh", [D, DH], F16, kind="ExternalInput").ap()
    # biases packed: [bz | bzn | bh] x NM m-tiles -> (128, 3*NM), one DMA
    bias = nc.dram_tensor("bias", [128, 3 * NM], F32, kind="ExternalInput").ap()
    hT = nc.dram_tensor("hT", [DH, SEQ], F32, kind="ExternalOutput").ap()

    with TileContext(nc) as tc:
        with (
            tc.tile_pool(name="weights", bufs=1) as wpool,
            tc.tile_pool(name="bias", bufs=1) as biaspool,
            tc.tile_pool(name="xt", bufs=4) as xpool,
            tc.tile_pool(name="a", bufs=4) as apool,
            tc.tile_pool(name="z", bufs=4) as zpool,
            tc.tile_pool(name="b", bufs=4) as bpool,
            tc.tile_pool(name="h", bufs=3) as hpool,
            tc.tile_pool(name="psz", bufs=4, space="PSUM") as pszpool,
            tc.tile_pool(name="psh", bufs=4, space="PSUM") as pshpool,
        ):
            # Weights resident for the whole kernel: (128 k, 512 m) per
            # k-tile. Weight/bias/output DMAs ride the SWDGE (gpsimd) path;
            # the sync HWDGE ring is dedicated to x^T prefetch and the
            # scalar ring stays free for ACT compute.
            wz_b, wh_b = [], []
            for kt in range(NK):
                w1 = wpool.tile([128, DH], F16, tag=f"wz{kt}")
                # k-tile 0 gates the first LDWEIGHTS: give it the
                # low-latency HWDGE sync ring ahead of the x^T stream.
                eng = nc.sync if kt == 0 else nc.gpsimd
                eng.dma_start(out=w1[:], in_=wz[kt * 128:(kt + 1) * 128, :])
                wz_b.append(w1)
            for kt in range(NK):
                w2 = wpool.tile([128, DH], F16, tag=f"wh{kt}")
                nc.gpsimd.dma_start(out=w2[:], in_=wh[kt * 128:(kt + 1) * 128, :])
                wh_b.append(w2)
            wz_t = [[wz_b[kt][:, m * 128:(m + 1) * 128] for m in range(NM)]
                    for kt in range(NK)]
            wh_t = [[wh_b[kt][:, m * 128:(m + 1) * 128] for m in range(NM)]
                    for kt in range(NK)]

            # Bias after weights: only needed by ACT (~16us in), while wz
            # gates the very first matmul.
            bias_t = biaspool.tile([128, 3 * NM], F32, tag="bias")
            nc.gpsimd.dma_start(out=bias_t[:], in_=bias[:])
            bz_t = [bias_t[:, m:m + 1] for m in range(NM)]
            bzn_t = [bias_t[:, NM + m:NM + m + 1] for m in range(NM)]
            bh_t = [bias_t[:, 2 * NM + m:2 * NM + m + 1] for m in range(NM)]

            last_h = [None] * NM
            seq_off = 0
            for c in range(NCHUNK):
                chunk = CHUNKS[c]
                xt = []
                for kt in range(NK):
                    t = xpool.tile([128, CHUNK_MAX], F16, tag=f"x{kt}")
                    # Split issue load across the sync HWDGE ring and the
                    # idle SWDGE path: ~0.6us of sequencer time per issue
                    # otherwise serializes 8-deep per chunk on sync.
                    eng = nc.sync if kt % 2 == 0 else nc.gpsimd
                    eng.dma_start(
                        out=t[:, :chunk],
                        in_=xT[kt * 128:(kt + 1) * 128,
                               seq_off:seq_off + chunk],
                    )
                    xt.append(t)

                h_big = []
                for m in range(NM):
                    h_m = hpool.tile([128, CHUNK_MAX], F32, tag=f"h{m}")
                    h_big.append(h_m)
                bounds = []
                acc = 0
                while acc < chunk:
                    bounds.append((acc, min(chunk, acc + 512)))
                    acc = min(chunk, acc + 512)
                for w0, w1 in bounds:
                    for m in range(NM):
                        psz = pszpool.tile([128, 512], F32)
                        psh = pshpool.tile([128, 512], F32)
                        for kt in range(NK):
                            nc.tensor.matmul(
                                psz[:, :w1 - w0],
                                wz_t[kt][m][:],
                                xt[kt][:, w0:w1],
                                start=(kt == 0),
                                stop=(kt == NK - 1),
                            )
                        for kt in range(NK):
                            nc.tensor.matmul(
                                psh[:, :w1 - w0],
                                wh_t[kt][m][:],
                                xt[kt][:, w0:w1],
                                start=(kt == 0),
                                stop=(kt == NK - 1),
                            )
                        # a = 1 - sigmoid(z_pre + bz) = sigmoid(-z_pre - bz)
                        a_t = apool.tile([128, 512], F32)
                        nc.scalar.activation(a_t[:, :w1 - w0], psz[:, :w1 - w0],
                                             AF.Sigmoid,
                                             bias=bzn_t[m][:], scale=-1.0)
                        z_t = zpool.tile([128, 512], F32)
                        nc.scalar.activation(z_t[:, :w1 - w0], psz[:, :w1 - w0],
                                             AF.Sigmoid,
                                             bias=bz_t[m][:], scale=1.0)
                        # b = (h_pre + bh) * z
                        b_t = bpool.tile([128, 512], F32)
                        nc.vector.scalar_tensor_tensor(
                            b_t[:, :w1 - w0], psh[:, :w1 - w0], bh_t[m][:],
                            z_t[:, :w1 - w0],
                            op0=OP.add, op1=OP.mult,
                        )
                        # h_t = a_t * h_{t-1} + b_t along seq
                        h_t = h_big[m][:, w0:w1]
                        init = 0.0 if last_h[m] is None else last_h[m][:, -1:]
                        nc.vector.tensor_tensor_scan(
                            h_t, a_t[:, :w1 - w0], b_t[:, :w1 - w0], init,
                            op0=OP.mult, op1=OP.add,
                        )
                        last_h[m] = h_t
                # Final chunk's outputs go via HWDGE (sync) -- the SWDGE
                # path adds a slow GpSimd drain right at the kernel tail --
                # and at s5 granularity so earlier pieces flush during the
                # last scans.
                if c == NCHUNK - 1:
                    # Spread final flushes over both HWDGE issuers so the
                    # issue latency doesn't serialize at the tail.
                    tail_eng = [nc.sync, nc.scalar, nc.sync, nc.scalar]
                    for mm in range(NM):
                        for w0, w1 in bounds:
                            tail_eng[mm].dma_start(
                                out=hT[mm * 128:(mm + 1) * 128,
                                       seq_off + w0:seq_off + w1],
                                in_=h_big[mm][:, w0:w1],
                            )
                else:
                    for mm in range(NM):
                        nc.gpsimd.dma_start(
                            out=hT[mm * 128:(mm + 1) * 128,
                                   seq_off:seq_off + chunk],
                            in_=h_big[mm][:, :chunk],
                        )
                seq_off += chunk
    _split_sync_waits(nc)
    return nc


_NC_CACHE = None


def _get_program():
    global _NC_CACHE
    if _NC_CACHE is None:
        _NC_CACHE = _build_program()
    return _NC_CACHE


def _make_in_maps(x, Wz, bz, Wh, bh):
    xT16 = [np.ascontiguousarray(x[b].T).astype(np.float16) for b in range(BATCH)]
    wzh = [np.ascontiguousarray(Wz[:, c * DH:(c + 1) * DH]).astype(np.float16)
           for c in range(2)]
    whh = [np.ascontiguousarray(Wh[:, c * DH:(c + 1) * DH]).astype(np.float16)
           for c in range(2)]
    # bias[p, m] = bz[m*128+p]; columns [0:NM]=bz, [NM:2NM]=-bz, [2NM:3NM]=bh
    biases = []
    for c in range(2):
        bzc = bz[c * DH:(c + 1) * DH].astype(np.float32).reshape(NM, 128).T
        bhc = bh[c * DH:(c + 1) * DH].astype(np.float32).reshape(NM, 128).T
        biases.append(np.ascontiguousarray(np.hstack([bzc, -bzc, bhc])))
    in_maps = []
    for i in range(N_CORES):
        b, c = i // 2, i % 2
        in_maps.append({
            "xT": xT16[b], "wz": wzh[c], "wh": whh[c], "bias": biases[c],
        })
    return in_maps


def _run(x, Wz, bz, Wh, bh, trace=False, trace_cores=None):
    import time

    nc = _get_program()
    in_maps = _make_in_maps(x, Wz, bz, Wh, bh)
    res = None
    for attempt in range(3):
        try:
            res = run_bass_kernel_spmd(
                nc, in_maps, list(range(N_CORES)),
                trace=trace, trace_cores=trace_cores,
            )
            break
        except Exception:
            # Transient NRT device errors have been observed on the first
            # execution after a fresh compile; retry.
            if attempt == 2:
                raise
            time.sleep(10)
    out = np.empty((BATCH, SEQ, D), dtype=np.float32)
    for i in range(N_CORES):
        b, c = i // 2, i % 2
        out[b, :, c * DH:(c + 1) * DH] = res.results[i]["hT"].T
    return out, res


def kernel(x, Wz, bz, Wh, bh):
    x = np.asarray(x, dtype=np.float32)
    Wz = np.asarray(Wz, dtype=np.float32)
    Wh = np.asarray(Wh, dtype=np.float32)
    bz = np.asarray(bz, dtype=np.float32)
    bh = np.asarray(bh, dtype=np.float32)
    out, _ = _run(x, Wz, bz, Wh, bh, trace=False)
    return out


# revision 61
# speedup vs baseline: 1.0432x; 1.0432x over previous
"""MinGRU layer kernel for 8 Trainium2 NeuronCores.

Problem: x (4, 8192, 1024) f32; Wz, Wh (1024, 1024); bz, bh (1024,)
    z = sigmoid(x @ Wz + bz); h_tilde = x @ Wh + bh
    h_t = (1 - z_t) * h_{t-1} + z_t * h_tilde_t   (scan over seq, h_{-1} = 0)

Sharding: 8 cores = 4 batches x 2 output-dim halves. The scan is
independent per (batch, dim), so each core owns a full-sequence scan for
one batch and 512 of the 1024 output dims -- no cross-core traffic.

Layout: host pre-transposes x to (d_in, seq) fp16 per batch. On device the
matmul keeps W stationary (lhsT = W tile, natural layout) and streams x^T,
producing (d_out, seq) tiles in PSUM -- exactly the layout
tensor_tensor_scan needs (scan runs along the free/seq axis, one recurrence
per partition/dim). ScalarE computes a = sigmoid(-(z_pre)) and
z = sigmoid(z_pre) straight out of PSUM; VectorE fuses b = (h_pre + bh) * z
and then runs the scan. Output h^T (512, 8192) f32 is written contiguously;
the host transposes back during the gather.
"""

import sys

if "/opt/trn_rl_repo" not in sys.path:
    sys.path.insert(0, "/opt/trn_rl_repo")

import numpy as np

from concourse import bass, mybir
from concourse.tile import TileContext
from concourse.bass_utils import run_bass_kernel_spmd

BATCH, SEQ, D = 4, 8192, 1024
DH = 512            # output dims per core
N_CORES = 8
# Seq chunk schedule: small chunks first so the PE starts on real work
# early (warms the HAM clock gate) and the consumer engines ramp before
# the PE hits full streaming rate.
CHUNKS = [256, 256, 512] + [1024] * 6 + [512, 256, 128, 128]
assert sum(CHUNKS) == SEQ
NCHUNK = len(CHUNKS)
CHUNK_MAX = max(CHUNKS)
NM = DH // 128      # output-dim tiles per core
NK = D // 128       # contraction tiles

F16 = mybir.dt.float16
F32 = mybir.dt.float32
AF = mybir.ActivationFunctionType
OP = mybir.AluOpType


_WAIT_LIMIT = 1  # this walrus build rejects multiple sem waits per instruction


def _split_sync_waits(nc):
    """Move excess semaphore waits (beyond _WAIT_LIMIT) off each instruction
    onto same-engine nops inserted immediately before it. Waits only gate
    execution, so hoisting some onto a preceding nop in the same engine
    stream is semantics-preserving."""
    import bass_rust

    n_extra = 0
    for fn in nc.m.functions:
        for blk in fn.blocks:
            insts = blk.instructions
            out = []
            for inst in insts:
                si = inst.sync_info
                if si is not None and si.on_wait and len(si.on_wait) > _WAIT_LIMIT:
                    waits = list(si.on_wait)
                    head, tail = waits[:-_WAIT_LIMIT], waits[-_WAIT_LIMIT:]
                    for j in range(0, len(head), _WAIT_LIMIT):
                        n_extra += 1
                        nop = bass_rust.InstNoOp(
                            name=f"{inst.name}-waitsplit{j}",
                            engine=inst.engine,
                            sync_info=type(si)(
                                on_wait=head[j:j + _WAIT_LIMIT], on_update=[]
                            ),
                            bass_nofuse=True,
                        )
                        nc.register_instruction(nop, overwrite=True)
                        out.append(nop)
                    si.on_wait = tail
                out.append(inst)
            if n_extra:
                blk.instructions = out
    return n_extra


def _build_program():
    nc = bass.Bass("TRN2", target_bir_lowering=False, debug=False)

    xT = nc.dram_tensor("xT", [D, SEQ], F16, kind="ExternalInput").ap()
    wz = nc.dram_tensor("wz", [D, DH], F16, kind="ExternalInput").ap()
    wh = nc.dram_tensor("wh", [D, DH], F16, kind="ExternalInput").ap()
    # biases packed: [bz | bzn | bh] x NM m-tiles -> (128, 3*NM), one DMA
    bias = nc.dram_tensor("bias", [128, 3 * NM], F32, kind="ExternalInput").ap()
    hT = nc.dram_tensor("hT", [DH, SEQ], F32, kind="ExternalOutput").ap()

    with TileContext(nc) as tc:
        with (
            tc.tile_pool(name="weights", bufs=1) as wpool,
            tc.tile_pool(name="bias", bufs=1) as biaspool,
            tc.tile_pool(name="xt", bufs=4) as xpool,
            tc.tile_pool(name="a", bufs=4) as apool,
            tc.tile_pool(name="z", bufs=4) as zpool,
            tc.tile_pool(name="b", bufs=4) as bpool,
            tc.tile_pool(name="h", bufs=3) as hpool,
            tc.tile_pool(name="psz", bufs=4, space="PSUM") as pszpool,
            tc.tile_pool(name="psh", bufs=4, space="PSUM") as pshpool,
        ):
            # Weights resident for the whole kernel: (128 k, 512 m) per
            # k-tile. Weight/bias/output DMAs ride the SWDGE (gpsimd) path;
            # the sync HWDGE ring is dedicated to x^T prefetch and the
            # scalar ring stays free for ACT compute.
            wz_b, wh_b = [], []
            for kt in range(NK):
                w1 = wpool.tile([128, DH], F16, tag=f"wz{kt}")
                # k-tile 0 gates the first LDWEIGHTS: give it the
                # low-latency HWDGE sync ring ahead of the x^T stream.
                eng = nc.sync if kt == 0 else nc.gpsimd
                eng.dma_start(out=w1[:], in_=wz[kt * 128:(kt + 1) * 128, :])
                wz_b.append(w1)
            for kt in range(NK):
                w2 = wpool.tile([128, DH], F16, tag=f"wh{kt}")
                nc.gpsimd.dma_start(out=w2[:], in_=wh[kt * 128:(kt + 1) * 128, :])
                wh_b.append(w2)
            wz_t = [[wz_b[kt][:, m * 128:(m + 1) * 128] for m in range(NM)]
                    for kt in range(NK)]
            wh_t = [[wh_b[kt][:, m * 128:(m + 1) * 128] for m in range(NM)]
                    for kt in range(NK)]

            # Bias after weights: only needed by ACT (~16us in), while wz
            # gates the very first matmul.
            bias_t = biaspool.tile([128, 3 * NM], F32, tag="bias")
            nc.gpsimd.dma_start(out=bias_t[:], in_=bias[:])
            bz_t = [bias_t[:, m:m + 1] for m in range(NM)]
            bzn_t = [bias_t[:, NM + m:NM + m + 1] for m in range(NM)]
            bh_t = [bias_t[:, 2 * NM + m:2 * NM + m + 1] for m in range(NM)]

            last_h = [None] * NM
            seq_off = 0
            for c in range(NCHUNK):
                chunk = CHUNKS[c]
                xt = []
                for kt in range(NK):
                    t = xpool.tile([128, CHUNK_MAX], F16, tag=f"x{kt}")
                    nc.sync.dma_start(
                        out=t[:, :chunk],
                        in_=xT[kt * 128:(kt + 1) * 128,
                               seq_off:seq_off + chunk],
                    )
                    xt.append(t)

                h_big = []
                for m in range(NM):
                    h_m = hpool.tile([128, CHUNK_MAX], F32, tag=f"h{m}")
                    h_big.append(h_m)
                bounds = []
                acc = 0
                while acc < chunk:
                    bounds.append((acc, min(chunk, acc + 512)))
                    acc = min(chunk, acc + 512)
                for w0, w1 in bounds:
                    for m in range(NM):
                        psz = pszpool.tile([128, 512], F32)
                        psh = pshpool.tile([128, 512], F32)
                        for kt in range(NK):
                            nc.tensor.matmul(
                                psz[:, :w1 - w0],
                                wz_t[kt][m][:],
                                xt[kt][:, w0:w1],
                                start=(kt == 0),
                                stop=(kt == NK - 1),
                            )
                        for kt in range(NK):
                            nc.tensor.matmul(
                                psh[:, :w1 - w0],
                                wh_t[kt][m][:],
                                xt[kt][:, w0:w1],
                                start=(kt == 0),
                                stop=(kt == NK - 1),
                            )
                        # a = 1 - sigmoid(z_pre + bz) = sigmoid(-z_pre - bz)
                        a_t = apool.tile([128, 512], F32)
                        nc.scalar.activation(a_t[:, :w1 - w0], psz[:, :w1 - w0],
                                             AF.Sigmoid,
                                             bias=bzn_t[m][:], scale=-1.0)
                        z_t = zpool.tile([128, 512], F32)
                        nc.scalar.activation(z_t[:, :w1 - w0], psz[:, :w1 - w0],
                                             AF.Sigmoid,
                                             bias=bz_t[m][:], scale=1.0)
                        # b = (h_pre + bh) * z
                        b_t = bpool.tile([128, 512], F32)
                        nc.vector.scalar_tensor_tensor(
                            b_t[:, :w1 - w0], psh[:, :w1 - w0], bh_t[m][:],
                            z_t[:, :w1 - w0],
                            op0=OP.add, op1=OP.mult,
                        )
                        # h_t = a_t * h_{t-1} + b_t along seq
                        h_t = h_big[m][:, w0:w1]
                        init = 0.0 if last_h[m] is None else last_h[m][:, -1:]
                        nc.vector.tensor_tensor_scan(
                            h_t, a_t[:, :w1 - w0], b_t[:, :w1 - w0], init,
                            op0=OP.mult, op1=OP.add,
                        )
                        last_h[m] = h_t
                # Final chunk's outputs go via HWDGE (sync) -- the SWDGE
                # path adds a slow GpSimd drain right at the kernel tail --
                # and at s5 granularity so earlier pieces flush during the
                # last scans.
                if c == NCHUNK - 1:
                    # Spread final flushes over both HWDGE issuers so the
                    # issue latency doesn't serialize at the tail.
                    tail_eng = [nc.sync, nc.scalar, nc.sync, nc.scalar]
                    for mm in range(NM):
                        for w0, w1 in bounds:
                            tail_eng[mm].dma_start(
                                out=hT[mm * 128:(mm + 1) * 128,
                                       seq_off + w0:seq_off + w1],
                                in_=h_big[mm][:, w0:w1],
                            )
                else:
                    for mm in range(NM):
                        nc.gpsimd.dma_start(
                            out=hT[mm * 128:(mm + 1) * 128,
                                   seq_off:seq_off + chunk],
                            in_=h_big[mm][:, :chunk],
                        )
                seq_off += chunk
    _split_sync_waits(nc)
    return nc


_NC_CACHE = None


def _get_program():
    global _NC_CACHE
    if _NC_CACHE is None:
        _NC_CACHE = _build_program()
    return _NC_CACHE


def _make_in_maps(x, Wz, bz, Wh, bh):
    xT16 = [np.ascontiguousarray(x[b].T).astype(np.float16) for b in range(BATCH)]
    wzh = [np.ascontiguousarray(Wz[:, c * DH:(c + 1) * DH]).astype(np.float16)
           for c in range(2)]
    whh = [np.ascontiguousarray(Wh[:, c * DH:(c + 1) * DH]).astype(np.float16)
           for c in range(2)]
    # bias[p, m] = bz[m*128+p]; columns [0:NM]=bz, [NM:2NM]=-bz, [2NM:3NM]=bh
    biases = []
    for c in range(2):
        bzc = bz[c * DH:(c + 1) * DH].astype(np.float32).reshape(NM, 128).T
        bhc = bh[c * DH:(c + 1) * DH].astype(np.float32).reshape(NM, 128).T
        biases.append(np.ascontiguousarray(np.hstack([bzc, -bzc, bhc])))
    in_maps = []
    for i in range(N_CORES):
        b, c = i // 2, i % 2
        in_maps.append({
            "xT": xT16[b], "wz": wzh[c], "wh": whh[c], "bias": biases[c],
        })
    return in_maps


def _run(x, Wz, bz, Wh, bh, trace=False, trace_cores=None):
    import time

    nc = _get_program()
    in_maps = _make_in_maps(x, Wz, bz, Wh, bh)
    res = None
    for attempt in range(3):
        try:
            res = run_bass_kernel_spmd(
                nc, in_maps, list(range(N_CORES)),
                trace=trace, trace_cores=trace_cores,
            )
            break
        except Exception:
            # Transient NRT device errors have been observed on the first
            # execution after a fresh compile; retry.
            if attempt == 2:
                raise
            time.sleep(10)
    out = np.empty((BATCH, SEQ, D), dtype=np.float32)
    for i in range(N_CORES):
        b, c = i // 2, i % 2
        out[b, :, c * DH:(c + 1) * DH] = res.results[i]["hT"].T
    return out, res


def kernel(x, Wz, bz, Wh, bh):
    x = np.asarray(x, dtype=np.float32)
    Wz = np.asarray(Wz, dtype=np.float32)
    Wh = np.asarray(Wh, dtype=np.float32)
    bz = np.asarray(bz, dtype=np.float32)
    bh = np.asarray(bh, dtype=np.float32)
    out, _ = _run(x, Wz, bz, Wh, bh, trace=False)
    return out


# revision 63
# speedup vs baseline: 1.0458x; 1.0025x over previous
"""MinGRU layer kernel for 8 Trainium2 NeuronCores.

Problem: x (4, 8192, 1024) f32; Wz, Wh (1024, 1024); bz, bh (1024,)
    z = sigmoid(x @ Wz + bz); h_tilde = x @ Wh + bh
    h_t = (1 - z_t) * h_{t-1} + z_t * h_tilde_t   (scan over seq, h_{-1} = 0)

Sharding: 8 cores = 4 batches x 2 output-dim halves. The scan is
independent per (batch, dim), so each core owns a full-sequence scan for
one batch and 512 of the 1024 output dims -- no cross-core traffic.

Layout: host pre-transposes x to (d_in, seq) fp16 per batch. On device the
matmul keeps W stationary (lhsT = W tile, natural layout) and streams x^T,
producing (d_out, seq) tiles in PSUM -- exactly the layout
tensor_tensor_scan needs (scan runs along the free/seq axis, one recurrence
per partition/dim). ScalarE computes a = sigmoid(-(z_pre)) and
z = sigmoid(z_pre) straight out of PSUM; VectorE fuses b = (h_pre + bh) * z
and then runs the scan. Output h^T (512, 8192) f32 is written contiguously;
the host transposes back during the gather.
"""

import sys

if "/opt/trn_rl_repo" not in sys.path:
    sys.path.insert(0, "/opt/trn_rl_repo")

import numpy as np

from concourse import bass, mybir
from concourse.tile import TileContext
from concourse.bass_utils import run_bass_kernel_spmd

BATCH, SEQ, D = 4, 8192, 1024
DH = 512            # output dims per core
N_CORES = 8
# Seq chunk schedule: small chunks first so the PE starts on real work
# early (warms the HAM clock gate) and the consumer engines ramp before
# the PE hits full streaming rate.
CHUNKS = [256, 256, 512] + [1024] * 6 + [512, 256, 128, 128]
assert sum(CHUNKS) == SEQ
NCHUNK = len(CHUNKS)
CHUNK_MAX = max(CHUNKS)
NM = DH // 128      # output-dim tiles per core
NK = D // 128       # contraction tiles

F16 = mybir.dt.float16
F32 = mybir.dt.float32
AF = mybir.ActivationFunctionType
OP = mybir.AluOpType


_WAIT_LIMIT = 1  # this walrus build rejects multiple sem waits per instruction


def _split_sync_waits(nc):
    """Move excess semaphore waits (beyond _WAIT_LIMIT) off each instruction
    onto same-engine nops inserted immediately before it. Waits only gate
    execution, so hoisting some onto a preceding nop in the same engine
    stream is semantics-preserving."""
    import bass_rust

    n_extra = 0
    for fn in nc.m.functions:
        for blk in fn.blocks:
            insts = blk.instructions
            out = []
            for inst in insts:
                si = inst.sync_info
                if si is not None and si.on_wait and len(si.on_wait) > _WAIT_LIMIT:
                    waits = list(si.on_wait)
                    head, tail = waits[:-_WAIT_LIMIT], waits[-_WAIT_LIMIT:]
                    for j in range(0, len(head), _WAIT_LIMIT):
                        n_extra += 1
                        nop = bass_rust.InstNoOp(
                            name=f"{inst.name}-waitsplit{j}",
                            engine=inst.engine,
                            sync_info=type(si)(
                                on_wait=head[j:j + _WAIT_LIMIT], on_update=[]
                            ),
                            bass_nofuse=True,
                        )
                        nc.register_instruction(nop, overwrite=True)
                        out.append(nop)
                    si.on_wait = tail
                out.append(inst)
            if n_extra:
                blk.instructions = out
    return n_extra


def _build_program():
    nc = bass.Bass("TRN2", target_bir_lowering=False, debug=False)

    xT = nc.dram_tensor("xT", [D, SEQ], F16, kind="ExternalInput").ap()
    wz = nc.dram_tensor("wz", [D, DH], F16, kind="ExternalInput").ap()
    wh = nc.dram_tensor("wh", [D, DH], F16, kind="ExternalInput").ap()
    # biases packed: [bz | bzn | bh] x NM m-tiles -> (128, 3*NM), one DMA
    bias = nc.dram_tensor("bias", [128, 3 * NM], F32, kind="ExternalInput").ap()
    hT = nc.dram_tensor("hT", [DH, SEQ], F32, kind="ExternalOutput").ap()

    with TileContext(nc) as tc:
        with (
            tc.tile_pool(name="weights", bufs=1) as wpool,
            tc.tile_pool(name="bias", bufs=1) as biaspool,
            tc.tile_pool(name="xt", bufs=4) as xpool,
            tc.tile_pool(name="a", bufs=4) as apool,
            tc.tile_pool(name="z", bufs=4) as zpool,
            tc.tile_pool(name="b", bufs=4) as bpool,
            tc.tile_pool(name="h", bufs=4) as hpool,
            tc.tile_pool(name="psz", bufs=4, space="PSUM") as pszpool,
            tc.tile_pool(name="psh", bufs=4, space="PSUM") as pshpool,
        ):
            # Weights resident for the whole kernel: (128 k, 512 m) per
            # k-tile. Weight/bias/output DMAs ride the SWDGE (gpsimd) path;
            # the sync HWDGE ring is dedicated to x^T prefetch and the
            # scalar ring stays free for ACT compute.
            wz_b, wh_b = [], []
            for kt in range(NK):
                w1 = wpool.tile([128, DH], F16, tag=f"wz{kt}")
                # k-tile 0 gates the first LDWEIGHTS: give it the
                # low-latency HWDGE sync ring ahead of the x^T stream.
                eng = nc.sync if kt == 0 else nc.gpsimd
                eng.dma_start(out=w1[:], in_=wz[kt * 128:(kt + 1) * 128, :])
                wz_b.append(w1)
            for kt in range(NK):
                w2 = wpool.tile([128, DH], F16, tag=f"wh{kt}")
                nc.gpsimd.dma_start(out=w2[:], in_=wh[kt * 128:(kt + 1) * 128, :])
                wh_b.append(w2)
            wz_t = [[wz_b[kt][:, m * 128:(m + 1) * 128] for m in range(NM)]
                    for kt in range(NK)]
            wh_t = [[wh_b[kt][:, m * 128:(m + 1) * 128] for m in range(NM)]
                    for kt in range(NK)]

            # Bias after weights: only needed by ACT (~16us in), while wz
            # gates the very first matmul.
            bias_t = biaspool.tile([128, 3 * NM], F32, tag="bias")
            nc.gpsimd.dma_start(out=bias_t[:], in_=bias[:])
            bz_t = [bias_t[:, m:m + 1] for m in range(NM)]
            bzn_t = [bias_t[:, NM + m:NM + m + 1] for m in range(NM)]
            bh_t = [bias_t[:, 2 * NM + m:2 * NM + m + 1] for m in range(NM)]

            last_h = [None] * NM
            seq_off = 0
            for c in range(NCHUNK):
                chunk = CHUNKS[c]
                xt = []
                for kt in range(NK):
                    t = xpool.tile([128, CHUNK_MAX], F16, tag=f"x{kt}")
                    nc.sync.dma_start(
                        out=t[:, :chunk],
                        in_=xT[kt * 128:(kt + 1) * 128,
                               seq_off:seq_off + chunk],
                    )
                    xt.append(t)

                h_big = []
                for m in range(NM):
                    h_m = hpool.tile([128, CHUNK_MAX], F32, tag=f"h{m}")
                    h_big.append(h_m)
                bounds = []
                acc = 0
                while acc < chunk:
                    bounds.append((acc, min(chunk, acc + 512)))
                    acc = min(chunk, acc + 512)
                for w0, w1 in bounds:
                    for m in range(NM):
                        psz = pszpool.tile([128, 512], F32)
                        psh = pshpool.tile([128, 512], F32)
                        for kt in range(NK):
                            nc.tensor.matmul(
                                psz[:, :w1 - w0],
                                wz_t[kt][m][:],
                                xt[kt][:, w0:w1],
                                start=(kt == 0),
                                stop=(kt == NK - 1),
                            )
                        for kt in range(NK):
                            nc.tensor.matmul(
                                psh[:, :w1 - w0],
                                wh_t[kt][m][:],
                                xt[kt][:, w0:w1],
                                start=(kt == 0),
                                stop=(kt == NK - 1),
                            )
                        # z first: the DVE multiply consumes it, so z-then-a
                        # shortens the STT->scan critical path by one ACT op.
                        z_t = zpool.tile([128, 512], F32)
                        nc.scalar.activation(z_t[:, :w1 - w0], psz[:, :w1 - w0],
                                             AF.Sigmoid,
                                             bias=bz_t[m][:], scale=1.0)
                        # a = 1 - sigmoid(z_pre + bz) = sigmoid(-z_pre - bz)
                        a_t = apool.tile([128, 512], F32)
                        nc.scalar.activation(a_t[:, :w1 - w0], psz[:, :w1 - w0],
                                             AF.Sigmoid,
                                             bias=bzn_t[m][:], scale=-1.0)
                        # b = (h_pre + bh) * z
                        b_t = bpool.tile([128, 512], F32)
                        nc.vector.scalar_tensor_tensor(
                            b_t[:, :w1 - w0], psh[:, :w1 - w0], bh_t[m][:],
                            z_t[:, :w1 - w0],
                            op0=OP.add, op1=OP.mult,
                        )
                        # h_t = a_t * h_{t-1} + b_t along seq
                        h_t = h_big[m][:, w0:w1]
                        init = 0.0 if last_h[m] is None else last_h[m][:, -1:]
                        nc.vector.tensor_tensor_scan(
                            h_t, a_t[:, :w1 - w0], b_t[:, :w1 - w0], init,
                            op0=OP.mult, op1=OP.add,
                        )
                        last_h[m] = h_t
                # Final chunk's outputs go via HWDGE (sync) -- the SWDGE
                # path adds a slow GpSimd drain right at the kernel tail --
                # and at s5 granularity so earlier pieces flush during the
                # last scans.
                if c == NCHUNK - 1:
                    # Spread final flushes over both HWDGE issuers so the
                    # issue latency doesn't serialize at the tail.
                    tail_eng = [nc.sync, nc.scalar, nc.sync, nc.scalar]
                    for mm in range(NM):
                        for w0, w1 in bounds:
                            tail_eng[mm].dma_start(
                                out=hT[mm * 128:(mm + 1) * 128,
                                       seq_off + w0:seq_off + w1],
                                in_=h_big[mm][:, w0:w1],
                            )
                else:
                    for mm in range(NM):
                        nc.gpsimd.dma_start(
                            out=hT[mm * 128:(mm + 1) * 128,
                                   seq_off:seq_off + chunk],
                            in_=h_big[mm][:, :chunk],
                        )
                seq_off += chunk
    _split_sync_waits(nc)
    return nc


_NC_CACHE = None


def _get_program():
    global _NC_CACHE
    if _NC_CACHE is None:
        _NC_CACHE = _build_program()
    return _NC_CACHE


def _make_in_maps(x, Wz, bz, Wh, bh):
    xT16 = [np.ascontiguousarray(x[b].T).astype(np.float16) for b in range(BATCH)]
    wzh = [np.ascontiguousarray(Wz[:, c * DH:(c + 1) * DH]).astype(np.float16)
           for c in range(2)]
    whh = [np.ascontiguousarray(Wh[:, c * DH:(c + 1) * DH]).astype(np.float16)
           for c in range(2)]
    # bias[p, m] = bz[m*128+p]; columns [0:NM]=bz, [NM:2NM]=-bz, [2NM:3NM]=bh
    biases = []
    for c in range(2):
        bzc = bz[c * DH:(c + 1) * DH].astype(np.float32).reshape(NM, 128).T
        bhc = bh[c * DH:(c + 1) * DH].astype(np.float32).reshape(NM, 128).T
        biases.append(np.ascontiguousarray(np.hstack([bzc, -bzc, bhc])))
    in_maps = []
    for i in range(N_CORES):
        b, c = i // 2, i % 2
        in_maps.append({
            "xT": xT16[b], "wz": wzh[c], "wh": whh[c], "bias": biases[c],
        })
    return in_maps


def _run(x, Wz, bz, Wh, bh, trace=False, trace_cores=None):
    import time

    nc = _get_program()
    in_maps = _make_in_maps(x, Wz, bz, Wh, bh)
    res = None
    for attempt in range(3):
        try:
            res = run_bass_kernel_spmd(
                nc, in_maps, list(range(N_CORES)),
                trace=trace, trace_cores=trace_cores,
            )
            break
        except Exception:
            # Transient NRT device errors have been observed on the first
            # execution after a fresh compile; retry.
            if attempt == 2:
                raise
            time.sleep(10)
    out = np.empty((BATCH, SEQ, D), dtype=np.float32)
    for i in range(N_CORES):
        b, c = i // 2, i % 2
        out[b, :, c * DH:(c + 1) * DH] = res.results[i]["hT"].T
    return out, res


def kernel(x, Wz, bz, Wh, bh):
    x = np.asarray(x, dtype=np.float32)
    Wz = np.asarray(Wz, dtype=np.float32)
    Wh = np.asarray(Wh, dtype=np.float32)
    bz = np.asarray(bz, dtype=np.float32)
    bh = np.asarray(bh, dtype=np.float32)
    out, _ = _run(x, Wz, bz, Wh, bh, trace=False)
    return out
